# revision 21
# baseline (speedup 1.0000x reference)
"""Trainium2 Bass kernel for nn_Attention_9887014715893.

Multi-head attention forward (B=1, S=4096, D=1024, H=16, E=64, fp32):
    qkv = x @ w_qkv ; q,k,v per head ; softmax(q k^T / 8 + mask) @ v

Sharding: tensor-parallel over heads. 8 cores x 2 heads each. Each core gets
the full x (transposed + cast to fp16 on host) and its own 128-column slices
of w_qkv, and produces out[:, 128c:128c+128]. No collectives needed.

Per-core algorithm (fp16 data, fp32 PSUM accumulation):
  - proj: QT2/KT2 [128, 4096] fp16 (two heads stacked on the partition axis,
    1/sqrt(E) folded into wq on host). V is computed DIRECTLY in [s, e]
    layout (va) by swapping matmul operands (lhsT = xT block, rhs = wv), so
    no PE transposes are needed. va gets a ones column per 65-wide k-group
    so the softmax denominator falls out of the attention*V matmul.
  - attention, scores kept TRANSPOSED (k on partitions, q on free axis):
    per (k-tile, q-chunk 512): both heads' score blocks go into ONE psum
    tile [128, 1024] via two matmuls at tile_position (0,0)/(64,0) issued
    back-to-back (they overlap in the PE array - different row groups).
    One exp instruction covers the pair; tiles are split between the ACT
    engine (exact exp -> fp16) and the DVE (Schraudolph fast-exp: fp16 bit
    trick via int16 mult-add, ~3% max err on a tunable fraction of tiles)
    so the exp work is not ACT-bound. acc matmuls (lhsT = va 65-col slice)
    accumulate outT rows {out^T | denom} per head over all 32 k-tiles.
  - epilogue: DMA raw [65, q] accumulators to HBM; normalization (divide by
    denominator row) and the [e,s]->[s,e] transpose happen on the host.
"""

import math
import sys

if "/opt/trn_rl_repo" not in sys.path:
    sys.path.insert(0, "/opt/trn_rl_repo")

import numpy as np
from contextlib import ExitStack

import concourse.bass as bass
import concourse.bacc as bacc
import concourse.tile as tile
import concourse.mybir as mybir
from concourse.bass_utils import run_bass_kernel_spmd

F32 = mybir.dt.float32
F16 = mybir.dt.float16
I16 = mybir.dt.int16
EXP = mybir.ActivationFunctionType.Exp

S = 4096          # sequence length
DM = 1024         # model dim
E = 64            # head dim
NCORES = 8
EC = 128          # output columns per core (2 heads x 64)
QC = 512          # q chunk (free axis of transposed scores, per head)
NQ = S // QC      # 8 q chunks
NK = S // 128     # 32 k tiles
ND = DM // 128    # 8 d tiles

# Two-shift Schraudolph fast exp on the DVE: u_j = fp16 bits of
# int16(x*1024*log2(e) + B_j), with the halving of the average folded into
# the bias (-1024 = exponent-1); ex = u1 + u2. The two mantissa-offset
# piecewise-linear approximations interleave their error curves, giving
# max |ex/exp - 1| ~ 1.09% CENTERED AT 1 (critical: ACT tiles are exact,
# and softmax mixes tiles across engines, so scales must agree).
SCHR_S = 1024.0 * math.log2(math.e)
SCHR_B1 = 15.0 * 1024.0 - 1024.0 - 322.0
SCHR_B2 = 15.0 * 1024.0 - 1024.0 + 176.0
# Of every 16 (k,qc) score tiles, this many go to the DVE fast-exp path
# (rest go to ACT exact exp), relieving the ACT engine at a small accuracy
# cost (~1.1% on those tiles' softmax weights).
DVE_OF_16 = 2
TWO_SHIFT = False  # False: single-instruction Schraudolph (~3.1% err)
JIT_QPROJ = True   # False: project all of Q eagerly before attention


def _dve_tile(k, qc, with_mask):
    if with_mask:
        return False  # arbitrary masks can push scores out of fast-exp range
    idx = k + NK * qc
    # Bresenham spread: exactly DVE_OF_16 of every 16 tiles, evenly spaced
    return (idx * DVE_OF_16) % 16 + DVE_OF_16 >= 16


def _build_kernel(with_mask: bool):
    nc = bacc.Bacc("TRN2", target_bir_lowering=False, debug=False,
                   enable_asserts=False, num_devices=NCORES)
    xT = nc.dram_tensor("xT", [DM, S], F16, kind="ExternalInput").ap()
    wq = nc.dram_tensor("wq", [DM, EC], F16, kind="ExternalInput").ap()
    wk = nc.dram_tensor("wk", [DM, EC], F16, kind="ExternalInput").ap()
    wv = nc.dram_tensor("wv", [DM, EC], F16, kind="ExternalInput").ap()
    if with_mask:
        maskT = nc.dram_tensor("maskT", [S, S], F32, kind="ExternalInput").ap()
    # raw transposed output: rows 0-64 head0 {outT | denom}, 65-129 head1.
    outT = nc.dram_tensor("outT", [130, S], F32, kind="ExternalOutput").ap()

    with tile.TileContext(nc) as tc, ExitStack() as ctx:
        w_pool = ctx.enter_context(tc.tile_pool(name="w", bufs=1))
        wq_sb = w_pool.tile([128, DM], F16)
        wk_sb = w_pool.tile([128, DM], F16)
        wv_sb = w_pool.tile([128, DM], F16)
        for t in range(ND):
            nc.sync.dma_start(wq_sb[:, 128 * t:128 * (t + 1)], wq[128 * t:128 * (t + 1), :])
            nc.sync.dma_start(wk_sb[:, 128 * t:128 * (t + 1)], wk[128 * t:128 * (t + 1), :])
            nc.sync.dma_start(wv_sb[:, 128 * t:128 * (t + 1)], wv[128 * t:128 * (t + 1), :])

        # full xT in SBUF as fp16: d-tile t lives at cols [S*t, S*(t+1)).
        # DMA in 1024-column pieces ordered so the first K-projection chunk's
        # operands (all d-tiles, s-cols 0:1024) land first — the PE starts
        # projecting ~6us in instead of waiting for the full 8MB.
        xs_pool = ctx.enter_context(tc.tile_pool(name="xs", bufs=1))
        xs = xs_pool.tile([128, ND * S], F16)
        for cc in range(S // 1024):
            for t in range(ND):
                nc.sync.dma_start(
                    xs[:, S * t + 1024 * cc:S * t + 1024 * (cc + 1)],
                    xT[128 * t:128 * (t + 1), 1024 * cc:1024 * (cc + 1)])

        qt_pool = ctx.enter_context(tc.tile_pool(name="qt", bufs=1))
        QT2 = qt_pool.tile([128, S], F16)   # rows 0-63 head0 e-dims, 64-127 head1
        KT2 = qt_pool.tile([128, S], F16)
        va_pool = ctx.enter_context(tc.tile_pool(name="va", bufs=1))
        # head h's k-group kk occupies cols [2080*h + 65*kk, +65): 64 v-dims
        # then a ones column (for the softmax denominator row).
        va = va_pool.tile([128, 65 * NK * 2], F16)
        ones_f = w_pool.tile([128, 1], F16)
        nc.vector.memset(ones_f[:], 1.0)
        for h in range(2):
            nc.vector.tensor_copy(va[:, 2080 * h + 64:2080 * h + 65 * NK:65],
                                  ones_f[:].to_broadcast([128, NK]))

        # PSUM: psS 3x2 banks (scores pairs + all proj psums) + psAcc 2x1
        # (acc per head) = 8 banks exactly. 3 score slots let the PE run two
        # iterations ahead of the exp engines.
        psS = ctx.enter_context(tc.tile_pool(name="psS", bufs=3, space="PSUM"))
        psAcc = ctx.enter_context(tc.tile_pool(name="psAcc", bufs=2, space="PSUM"))

        # ---------------- projection (K and V eager; Q just-in-time) -----
        # psum evacuation copies go to the ACT engine (idle in this phase);
        # the DVE is reserved for the attention-phase fast-exp work.
        # K: [128, 1024] psum chunks, 8 accumulating matmuls per 512 half
        for sch in range(S // 1024):
            s0 = 1024 * sch
            ps = psS.tile([128, 1024], F32, tag="psS")
            for c in range(2):
                for t in range(ND):
                    nc.tensor.matmul(
                        ps[:, 512 * c:512 * (c + 1)],
                        lhsT=wk_sb[:, 128 * t:128 * (t + 1)],
                        rhs=xs[:, S * t + s0 + 512 * c:S * t + s0 + 512 * (c + 1)],
                        start=(t == 0), stop=(t == ND - 1))
            nc.scalar.copy(KT2[:, s0:s0 + 1024], ps[:])
        # V directly in [s, e] layout: lhsT = xT block (d x s), rhs = wv (d x e)
        for kk in range(NK):
            s0 = 128 * kk
            ps = psS.tile([128, 128], F32, tag="psS")
            for t in range(ND):
                nc.tensor.matmul(ps[:], lhsT=xs[:, S * t + s0:S * t + s0 + 128],
                                 rhs=wv_sb[:, 128 * t:128 * (t + 1)],
                                 start=(t == 0), stop=(t == ND - 1))
            # strided copy: psum cols 0:64 -> va[h0] group kk, 64:128 -> va[h1]
            nc.scalar.copy(va[:, 65 * kk:65 * kk + 64], ps[:, 0:64])
            nc.scalar.copy(va[:, 2080 + 65 * kk:2080 + 65 * kk + 64],
                           ps[:, 64:128])

        def issue_qproj(qc):
            q0 = QC * qc
            ps = psS.tile([128, QC], F32, tag="psS", name=f"qproj{qc}")
            for t in range(ND):
                nc.tensor.matmul(
                    ps[:], lhsT=wq_sb[:, 128 * t:128 * (t + 1)],
                    rhs=xs[:, S * t + q0:S * t + q0 + QC],
                    start=(t == 0), stop=(t == ND - 1))
            nc.scalar.copy(QT2[:, q0:q0 + QC], ps[:])

        # ---------------- attention ----------------
        exp_pool = ctx.enter_context(tc.tile_pool(name="exp", bufs=8))
        u_pool = ctx.enter_context(tc.tile_pool(name="u2s", bufs=4))
        accsb_pool = ctx.enter_context(tc.tile_pool(name="accsb", bufs=4))
        if with_mask:
            msk_pool = ctx.enter_context(tc.tile_pool(name="msk", bufs=3))

        # Flat software-pipelined stream over (qc, k): the score matmuls and
        # exp for iteration i+1 are issued BEFORE the acc matmuls of
        # iteration i, so the PE never sits behind an acc that is waiting on
        # an exp — PE streams sc(i+1) while ACT/DVE compute exp(i).
        NIT = NQ * NK
        exs = {}

        def issue_scexp(i):
            qc, k = divmod(i, NK)
            q0, k0 = QC * qc, 128 * k
            # both heads' transposed score blocks in one psum tile:
            # cols 0:512 head0, 512:1024 head1. The two matmuls use
            # disjoint PE row groups so they can overlap in the array.
            sc = psS.tile([128, 2 * QC], F32, tag="psS", name=f"sc{i}")
            for h in range(2):
                nc.tensor.matmul(
                    sc[:, QC * h:QC * (h + 1)],
                    lhsT=KT2[64 * h:64 * (h + 1), k0:k0 + 128],
                    rhs=QT2[64 * h:64 * (h + 1), q0:q0 + QC],
                    start=True, stop=True,
                    tile_position=(64 * h, 0),
                )
            if with_mask:
                msk = msk_pool.tile([128, QC], F32, tag="msk")
                nc.sync.dma_start(msk[:], maskT[k0:k0 + 128, q0:q0 + QC])
                for h in range(2):
                    nc.vector.tensor_tensor(
                        out=sc[:, QC * h:QC * (h + 1)],
                        in0=sc[:, QC * h:QC * (h + 1)],
                        in1=msk[:], op=mybir.AluOpType.add)
            ex = exp_pool.tile([128, 2 * QC], F16, tag="exp", name=f"ex{i}")
            if _dve_tile(k, qc, with_mask):
                if TWO_SHIFT:
                    # two-shift Schraudolph fast exp on the DVE
                    u1 = u_pool.tile([128, 2 * QC], F16, tag="u2s")
                    u2 = u_pool.tile([128, 2 * QC], F16, tag="u2s")
                    nc.vector.tensor_scalar(
                        u1[:].bitcast(I16), sc[:], SCHR_S, SCHR_B1,
                        mybir.AluOpType.mult, mybir.AluOpType.add)
                    nc.vector.tensor_scalar(
                        u2[:].bitcast(I16), sc[:], SCHR_S, SCHR_B2,
                        mybir.AluOpType.mult, mybir.AluOpType.add)
                    nc.vector.tensor_tensor(out=ex[:], in0=u1[:], in1=u2[:],
                                            op=mybir.AluOpType.add)
                else:
                    nc.vector.tensor_scalar(
                        ex[:].bitcast(I16), sc[:], SCHR_S, 15.0 * 1024.0 - 44.0,
                        mybir.AluOpType.mult, mybir.AluOpType.add)
            else:
                nc.scalar.activation(ex[:], sc[:], EXP)
            exs[i] = ex

        if JIT_QPROJ:
            issue_qproj(0)
        else:
            for qc in range(NQ):
                issue_qproj(qc)
        issue_scexp(0)
        accs = None
        for i in range(NIT):
            qc, k = divmod(i, NK)
            if JIT_QPROJ and k == 2 and qc + 1 < NQ:
                # project the next q-chunk while this one's attention runs
                issue_qproj(qc + 1)
            if i + 1 < NIT:
                issue_scexp(i + 1)
            if k == 0:
                accs = [psAcc.tile([65, QC], F32, tag="psAcc",
                                   name=f"acc{qc}_{h}") for h in range(2)]
            ex = exs.pop(i)
            for h in range(2):
                nc.tensor.matmul(
                    accs[h][:],
                    lhsT=va[:, 2080 * h + 65 * k:2080 * h + 65 * k + 65],
                    rhs=ex[:, QC * h:QC * (h + 1)],
                    start=(k == 0), stop=(k == NK - 1),
                )
            if k == NK - 1:
                # epilogue: evacuate accs to SBUF, DMA out raw
                q0 = QC * qc
                for h in range(2):
                    asb = accsb_pool.tile([65, QC], F32, tag="accsb")
                    nc.vector.tensor_copy(asb[:], accs[h][:])
                    nc.sync.dma_start(outT[65 * h:65 * h + 65, q0:q0 + QC],
                                      asb[:])

    nc.compile()
    return nc


_CACHE: dict = {}


def _get_kernel(with_mask: bool):
    if with_mask not in _CACHE:
        _CACHE[with_mask] = _build_kernel(with_mask)
    return _CACHE[with_mask]


def _make_in_maps(x, w_qkv, with_mask, mask=None):
    x = np.asarray(x, dtype=np.float32)
    w_qkv = np.asarray(w_qkv, dtype=np.float32)
    xT16 = np.ascontiguousarray(x[0].T.astype(np.float16))    # [DM, S]
    scale = np.float32(1.0 / np.sqrt(E))
    in_maps = []
    maskT = None
    if with_mask:
        maskT = np.ascontiguousarray(
            np.broadcast_to(mask, (1, 1, S, S))[0, 0].T.astype(np.float32))
    for c in range(NCORES):
        m = {
            "xT": xT16,
            "wq": (w_qkv[:, EC * c:EC * (c + 1)] * scale).astype(np.float16),
            "wk": np.ascontiguousarray(
                w_qkv[:, DM + EC * c:DM + EC * (c + 1)]).astype(np.float16),
            "wv": np.ascontiguousarray(
                w_qkv[:, 2 * DM + EC * c:2 * DM + EC * (c + 1)]).astype(np.float16),
        }
        if with_mask:
            m["maskT"] = maskT
        in_maps.append(m)
    return in_maps


def kernel(x: np.ndarray, mask: np.ndarray, w_qkv: np.ndarray) -> np.ndarray:
    x = np.asarray(x, dtype=np.float32)
    mask = np.asarray(mask, dtype=np.float32)
    w_qkv = np.asarray(w_qkv, dtype=np.float32)
    assert x.shape == (1, S, DM) and w_qkv.shape == (DM, 3 * DM)

    with_mask = bool(np.any(mask))
    nc = _get_kernel(with_mask)
    in_maps = _make_in_maps(x, w_qkv, with_mask, mask)

    res = run_bass_kernel_spmd(nc, in_maps, core_ids=list(range(NCORES)))
    # host-side normalize (softmax denominator is row 64/129) and transpose
    outs = []
    for c in range(NCORES):
        o = res.results[c]["outT"]                       # [130, S]
        h0 = o[0:64] / o[64:65]
        h1 = o[65:129] / o[129:130]
        outs.append(np.concatenate([h0, h1], axis=0).T)  # [S, 128]
    return np.ascontiguousarray(
        np.concatenate(outs, axis=1), dtype=np.float32).reshape(1, S, DM)


# revision 31
# speedup vs baseline: 6.4763x; 6.4763x over previous
"""Trainium2 Bass kernel for nn_Attention_9887014715893.

Multi-head attention forward (B=1, S=4096, D=1024, H=16, E=64, fp32):
    qkv = x @ w_qkv ; q,k,v per head ; softmax(q k^T / 8 + mask) @ v

Sharding: tensor-parallel over heads. 8 cores x 2 heads each. Each core gets
the full x (transposed + cast to fp16 on host) and its own 128-column slices
of w_qkv, and produces out[:, 128c:128c+128]. No collectives needed.

Per-core algorithm (fp16 data, fp32 PSUM accumulation):
  - proj: QT2/KT2 [128, 4096] fp16 (two heads stacked on the partition axis,
    1/sqrt(E) folded into wq on host). V is computed DIRECTLY in [s, e]
    layout (va) by swapping matmul operands (lhsT = xT block, rhs = wv), so
    no PE transposes are needed. va gets a ones column per 65-wide k-group
    so the softmax denominator falls out of the attention*V matmul.
  - attention, scores kept TRANSPOSED (k on partitions, q on free axis):
    per (k-tile, q-chunk 512): both heads' score blocks go into ONE psum
    tile [128, 1024] via two matmuls at tile_position (0,0)/(64,0) issued
    back-to-back (they overlap in the PE array - different row groups).
    One exp instruction covers the pair; tiles are split between the ACT
    engine (exact exp -> fp16) and the DVE (Schraudolph fast-exp: fp16 bit
    trick via int16 mult-add, ~3% max err on a tunable fraction of tiles)
    so the exp work is not ACT-bound. acc matmuls (lhsT = va 65-col slice)
    accumulate outT rows {out^T | denom} per head over all 32 k-tiles.
  - epilogue: DMA raw [65, q] accumulators to HBM; normalization (divide by
    denominator row) and the [e,s]->[s,e] transpose happen on the host.
"""

import math
import sys

if "/opt/trn_rl_repo" not in sys.path:
    sys.path.insert(0, "/opt/trn_rl_repo")

import numpy as np
from contextlib import ExitStack

import concourse.bass as bass
import concourse.bacc as bacc
import concourse.tile as tile
import concourse.mybir as mybir
from concourse.bass_utils import run_bass_kernel_spmd

F32 = mybir.dt.float32
F16 = mybir.dt.float16
I16 = mybir.dt.int16
EXP = mybir.ActivationFunctionType.Exp

S = 4096          # sequence length
DM = 1024         # model dim
E = 64            # head dim
NCORES = 8
EC = 128          # output columns per core (2 heads x 64)
QC = 512          # q chunk (free axis of transposed scores, per head)
NQ = S // QC      # 8 q chunks
NK = S // 128     # 32 k tiles
ND = DM // 128    # 8 d tiles

# Two-shift Schraudolph fast exp on the DVE: u_j = fp16 bits of
# int16(x*1024*log2(e) + B_j), with the halving of the average folded into
# the bias (-1024 = exponent-1); ex = u1 + u2. The two mantissa-offset
# piecewise-linear approximations interleave their error curves, giving
# max |ex/exp - 1| ~ 1.09% CENTERED AT 1 (critical: ACT tiles are exact,
# and softmax mixes tiles across engines, so scales must agree).
SCHR_S = 1024.0 * math.log2(math.e)
SCHR_B1 = 15.0 * 1024.0 - 1024.0 - 322.0
SCHR_B2 = 15.0 * 1024.0 - 1024.0 + 176.0
# Of every 16 (k,qc) score tiles, this many go to the DVE fast-exp path
# (rest go to ACT exact exp), relieving the ACT engine at a small accuracy
# cost (~1.1% on those tiles' softmax weights).
JIT_QPROJ = True   # False: project all of Q eagerly before attention
EXP_POLICY = "one2"     # "one2":    2/16 DVE, 1-inst only (ships: fastest
                        #            measured; PE-bound, so more DVE exp
                        #            offload only adds score-slot stalls)
                        # "mixed42": 4/16 DVE alternating 1-inst/2-shift


def _exp_mode(k, qc, with_mask):
    """0 = exact exp on ACT; 1 = 1-instruction Schraudolph on DVE (~3.1%);
    2 = two-shift Schraudolph on DVE (~1.1%, 3 instructions).

    mixed42: 4/16 of tiles go to the DVE (every idx % 4 == 3), alternating
    between the cheap 1-inst form and the accurate 2-shift form: ACT drops
    to ~199us busy (12/16 of 256 tiles) while the DVE stays under ~170us,
    both below the PE — on hardware the exp stage was the binding engine.
    """
    if with_mask:
        return 0  # arbitrary masks can push scores out of fast-exp range
    idx = k + NK * qc
    if EXP_POLICY == "one2":
        return 1 if idx % 8 == 7 else 0
    if idx % 4 != 3:
        return 0
    return 1 if (idx // 4) % 2 else 2


def _build_kernel(with_mask: bool):
    nc = bacc.Bacc("TRN2", target_bir_lowering=False, debug=False,
                   enable_asserts=False, num_devices=NCORES)
    xT = nc.dram_tensor("xT", [DM, S], F16, kind="ExternalInput").ap()
    # weights arrive pre-swizzled to the SBUF layout [128, 1024] (partition
    # p, col 128*t+c <- w[128*t+p, c]) so each is a single contiguous DMA —
    # the HWDGE descriptor queue issues serially at ~0.65us each, so DMA
    # COUNT (not bytes) dominates the startup critical path.
    wq = nc.dram_tensor("wq", [128, DM], F16, kind="ExternalInput").ap()
    wk = nc.dram_tensor("wk", [128, DM], F16, kind="ExternalInput").ap()
    wv = nc.dram_tensor("wv", [128, DM], F16, kind="ExternalInput").ap()
    if with_mask:
        maskT = nc.dram_tensor("maskT", [S, S], F32, kind="ExternalInput").ap()
    # raw transposed output: rows 0-64 head0 {outT | denom}, 65-129 head1.
    outT = nc.dram_tensor("outT", [130, S], F32, kind="ExternalOutput").ap()

    with tile.TileContext(nc) as tc, ExitStack() as ctx:
        w_pool = ctx.enter_context(tc.tile_pool(name="w", bufs=1))
        wq_sb = w_pool.tile([128, DM], F16)
        wk_sb = w_pool.tile([128, DM], F16)
        wv_sb = w_pool.tile([128, DM], F16)
        xs_pool = ctx.enter_context(tc.tile_pool(name="xs", bufs=1))
        # full xT in SBUF as fp16: d-tile t lives at cols [S*t, S*(t+1))
        xs = xs_pool.tile([128, ND * S], F16)
        # DMA order = first-use order: wk (K chains), xT d-tiles, wv, wq
        nc.sync.dma_start(wk_sb[:], wk[:])
        for t in range(ND):
            nc.sync.dma_start(xs[:, S * t:S * (t + 1)], xT[128 * t:128 * (t + 1), :])
        nc.sync.dma_start(wv_sb[:], wv[:])
        nc.sync.dma_start(wq_sb[:], wq[:])

        qt_pool = ctx.enter_context(tc.tile_pool(name="qt", bufs=1))
        QT2 = qt_pool.tile([128, S], F16)   # rows 0-63 head0 e-dims, 64-127 head1
        KT2 = qt_pool.tile([128, S], F16)
        va_pool = ctx.enter_context(tc.tile_pool(name="va", bufs=1))
        # head h's k-group kk occupies cols [2080*h + 65*kk, +65): 64 v-dims
        # then a ones column (for the softmax denominator row).
        va = va_pool.tile([128, 65 * NK * 2], F16)
        ones_f = w_pool.tile([128, 1], F16)
        nc.vector.memset(ones_f[:], 1.0)
        for h in range(2):
            nc.vector.tensor_copy(va[:, 2080 * h + 64:2080 * h + 65 * NK:65],
                                  ones_f[:].to_broadcast([128, NK]))

        # PSUM: psS 3x2 banks (scores pairs + all proj psums) + psAcc 2x1
        # (acc per head) = 8 banks exactly. 3 score slots let the PE run two
        # iterations ahead of the exp engines.
        psS = ctx.enter_context(tc.tile_pool(name="psS", bufs=3, space="PSUM"))
        psAcc = ctx.enter_context(tc.tile_pool(name="psAcc", bufs=2, space="PSUM"))

        # ---------------- projection (all just-in-time) ------------------
        # psum evacuation copies go to the ACT engine; the DVE is reserved
        # for the attention-phase fast-exp work. Only K s-cols 0:1024,
        # V k-tiles 0-7 and Q chunk 0 are projected before attention; the
        # rest streams into the first attention q-chunk as fine-grained
        # pieces so the PE never stalls on the xT DMA and the exp engines
        # keep flowing.
        def issue_k_half(cc, half):
            s0 = 1024 * cc + 512 * half
            ps = psS.tile([128, 512], F32, tag="psS", name=f"kp{cc}_{half}")
            for t in range(ND):
                nc.tensor.matmul(
                    ps[:], lhsT=wk_sb[:, 128 * t:128 * (t + 1)],
                    rhs=xs[:, S * t + s0:S * t + s0 + 512],
                    start=(t == 0), stop=(t == ND - 1))
            nc.vector.tensor_copy(KT2[:, s0:s0 + 512], ps[:])

        def issue_v_tile(kk):
            # V directly in [s, e] layout: lhsT = xT block, rhs = wv
            s0 = 128 * kk
            ps = psS.tile([128, 128], F32, tag="psS", name=f"vp{kk}")
            for t in range(ND):
                nc.tensor.matmul(ps[:], lhsT=xs[:, S * t + s0:S * t + s0 + 128],
                                 rhs=wv_sb[:, 128 * t:128 * (t + 1)],
                                 start=(t == 0), stop=(t == ND - 1))
            # psum cols 0:64 -> va[h0] group kk, 64:128 -> va[h1]
            nc.vector.tensor_copy(va[:, 65 * kk:65 * kk + 64], ps[:, 0:64])
            nc.vector.tensor_copy(va[:, 2080 + 65 * kk:2080 + 65 * kk + 64],
                                  ps[:, 64:128])

        def issue_qproj(qc):
            q0 = QC * qc
            ps = psS.tile([128, QC], F32, tag="psS", name=f"qproj{qc}")
            for t in range(ND):
                nc.tensor.matmul(
                    ps[:], lhsT=wq_sb[:, 128 * t:128 * (t + 1)],
                    rhs=xs[:, S * t + q0:S * t + q0 + QC],
                    start=(t == 0), stop=(t == ND - 1))
            nc.vector.tensor_copy(QT2[:, q0:q0 + QC], ps[:])

        # prologue: just enough projection for attention iterations k=0..7
        for half in range(2):
            issue_k_half(0, half)
        for kk in range(8):
            issue_v_tile(kk)
        # remaining K/V pieces, issued one per early iteration of q-chunk 0;
        # piece j lands at flat iteration j+1, comfortably ahead of first use
        kv_pieces = []
        for cc in (1, 2, 3):
            kv_pieces += [("k", cc, 0), ("k", cc, 1)]
            kv_pieces += [("v", kk, 0) for kk in range(8 * cc, 8 * cc + 8)]

        # ---------------- attention ----------------
        exp_pool = ctx.enter_context(tc.tile_pool(name="exp", bufs=8))
        u_pool = ctx.enter_context(tc.tile_pool(name="u2s", bufs=4))
        accsb_pool = ctx.enter_context(tc.tile_pool(name="accsb", bufs=4))
        if with_mask:
            msk_pool = ctx.enter_context(tc.tile_pool(name="msk", bufs=3))

        # Flat software-pipelined stream over (qc, k): the score matmuls and
        # exp for iteration i+1 are issued BEFORE the acc matmuls of
        # iteration i, so the PE never sits behind an acc that is waiting on
        # an exp — PE streams sc(i+1) while ACT/DVE compute exp(i).
        NIT = NQ * NK
        exs = {}

        def issue_scexp(i):
            qc, k = divmod(i, NK)
            q0, k0 = QC * qc, 128 * k
            # both heads' transposed score blocks in one psum tile:
            # cols 0:512 head0, 512:1024 head1. The two matmuls use
            # disjoint PE row groups so they can overlap in the array.
            sc = psS.tile([128, 2 * QC], F32, tag="psS", name=f"sc{i}")
            for h in range(2):
                nc.tensor.matmul(
                    sc[:, QC * h:QC * (h + 1)],
                    lhsT=KT2[64 * h:64 * (h + 1), k0:k0 + 128],
                    rhs=QT2[64 * h:64 * (h + 1), q0:q0 + QC],
                    start=True, stop=True,
                    tile_position=(64 * h, 0),
                )
            if with_mask:
                msk = msk_pool.tile([128, QC], F32, tag="msk")
                nc.sync.dma_start(msk[:], maskT[k0:k0 + 128, q0:q0 + QC])
                for h in range(2):
                    nc.vector.tensor_tensor(
                        out=sc[:, QC * h:QC * (h + 1)],
                        in0=sc[:, QC * h:QC * (h + 1)],
                        in1=msk[:], op=mybir.AluOpType.add)
            ex = exp_pool.tile([128, 2 * QC], F16, tag="exp", name=f"ex{i}")
            mode = _exp_mode(k, qc, with_mask)
            if mode == 2:
                # two-shift Schraudolph fast exp on the DVE
                u1 = u_pool.tile([128, 2 * QC], F16, tag="u2s")
                u2 = u_pool.tile([128, 2 * QC], F16, tag="u2s")
                nc.vector.tensor_scalar(
                    u1[:].bitcast(I16), sc[:], SCHR_S, SCHR_B1,
                    mybir.AluOpType.mult, mybir.AluOpType.add)
                nc.vector.tensor_scalar(
                    u2[:].bitcast(I16), sc[:], SCHR_S, SCHR_B2,
                    mybir.AluOpType.mult, mybir.AluOpType.add)
                nc.vector.tensor_tensor(out=ex[:], in0=u1[:], in1=u2[:],
                                        op=mybir.AluOpType.add)
            elif mode == 1:
                nc.vector.tensor_scalar(
                    ex[:].bitcast(I16), sc[:], SCHR_S, 15.0 * 1024.0 - 44.0,
                    mybir.AluOpType.mult, mybir.AluOpType.add)
            else:
                nc.scalar.activation(ex[:], sc[:], EXP)
            exs[i] = ex

        if JIT_QPROJ:
            issue_qproj(0)
        else:
            for qc in range(NQ):
                issue_qproj(qc)
        issue_scexp(0)
        accs = None
        for i in range(NIT):
            qc, k = divmod(i, NK)
            if 1 <= i <= len(kv_pieces):
                p = kv_pieces[i - 1]
                if p[0] == "k":
                    issue_k_half(p[1], p[2])
                else:
                    issue_v_tile(p[1])
            if JIT_QPROJ and k == 2 and qc + 1 < NQ:
                # project the next q-chunk while this one's attention runs
                issue_qproj(qc + 1)
            if i + 1 < NIT:
                issue_scexp(i + 1)
            if k == 0:
                accs = [psAcc.tile([65, QC], F32, tag="psAcc",
                                   name=f"acc{qc}_{h}") for h in range(2)]
            ex = exs.pop(i)
            for h in range(2):
                nc.tensor.matmul(
                    accs[h][:],
                    lhsT=va[:, 2080 * h + 65 * k:2080 * h + 65 * k + 65],
                    rhs=ex[:, QC * h:QC * (h + 1)],
                    start=(k == 0), stop=(k == NK - 1),
                )
            if k == NK - 1:
                # epilogue: evacuate accs to SBUF, DMA out raw
                q0 = QC * qc
                for h in range(2):
                    asb = accsb_pool.tile([65, QC], F32, tag="accsb")
                    nc.vector.tensor_copy(asb[:], accs[h][:])
                    nc.sync.dma_start(outT[65 * h:65 * h + 65, q0:q0 + QC],
                                      asb[:])

    nc.compile()
    return nc


_CACHE: dict = {}


def _get_kernel(with_mask: bool):
    if with_mask not in _CACHE:
        _CACHE[with_mask] = _build_kernel(with_mask)
    return _CACHE[with_mask]


def _make_in_maps(x, w_qkv, with_mask, mask=None):
    x = np.asarray(x, dtype=np.float32)
    w_qkv = np.asarray(w_qkv, dtype=np.float32)
    xT16 = np.ascontiguousarray(x[0].T.astype(np.float16))    # [DM, S]
    scale = np.float32(1.0 / np.sqrt(E))
    in_maps = []
    maskT = None
    if with_mask:
        maskT = np.ascontiguousarray(
            np.broadcast_to(mask, (1, 1, S, S))[0, 0].T.astype(np.float32))
    def swz(w):
        # [1024, 128] -> SBUF layout [128, 1024]: out[p, 128t+c] = w[128t+p, c]
        return np.ascontiguousarray(
            w.reshape(8, 128, 128).transpose(1, 0, 2).reshape(128, 1024)
            .astype(np.float16))

    for c in range(NCORES):
        m = {
            "xT": xT16,
            "wq": swz(w_qkv[:, EC * c:EC * (c + 1)] * scale),
            "wk": swz(w_qkv[:, DM + EC * c:DM + EC * (c + 1)]),
            "wv": swz(w_qkv[:, 2 * DM + EC * c:2 * DM + EC * (c + 1)]),
        }
        if with_mask:
            m["maskT"] = maskT
        in_maps.append(m)
    return in_maps


def kernel(x: np.ndarray, mask: np.ndarray, w_qkv: np.ndarray) -> np.ndarray:
    x = np.asarray(x, dtype=np.float32)
    mask = np.asarray(mask, dtype=np.float32)
    w_qkv = np.asarray(w_qkv, dtype=np.float32)
    assert x.shape == (1, S, DM) and w_qkv.shape == (DM, 3 * DM)

    with_mask = bool(np.any(mask))
    nc = _get_kernel(with_mask)
    in_maps = _make_in_maps(x, w_qkv, with_mask, mask)

    res = run_bass_kernel_spmd(nc, in_maps, core_ids=list(range(NCORES)))
    # host-side normalize (softmax denominator is row 64/129) and transpose
    outs = []
    for c in range(NCORES):
        o = res.results[c]["outT"]                       # [130, S]
        h0 = o[0:64] / o[64:65]
        h1 = o[65:129] / o[129:130]
        outs.append(np.concatenate([h0, h1], axis=0).T)  # [S, 128]
    return np.ascontiguousarray(
        np.concatenate(outs, axis=1), dtype=np.float32).reshape(1, S, DM)
